# revision 1
# baseline (speedup 1.0000x reference)
"""Causal self-attention (B=4, S=2048, D=1024, single 1024-wide head) on 8 TRN2 cores.

Sharding: core c -> batch b=c//2, parity h=c%2. Each core computes K/V for its
whole batch (duplicated across the 2 cores of a batch) and handles the 8
query blocks {h, h+2, ..., h+14} (128 rows each). Pairing strided blocks keeps
causal work balanced and — with key-extents padded to 256*(j+1) — makes the
program identical on every core; causality differences live in per-core
additive-mask input data, not control flow.

All matmuls run on the PE in bf16 with fp32 PSUM accumulation. Softmax skips
max-subtraction (scores are ~N(0,1); exp stays in fp32 range) so the
denominator comes free from the Exp activation's accumulate output.
"""

import numpy as np
import ml_dtypes

import concourse.bass as bass
import concourse.bacc as bacc
import concourse.tile as tile
from concourse import mybir
from concourse import bass_utils
from concourse.masks import make_identity

BF16 = ml_dtypes.bfloat16
P = 128
B, S, D = 4, 2048, 1024
EC = D // P  # contraction chunks (8)
NQB = 8      # query blocks per core
NKB = S // P  # key blocks per batch (16)
NCORES = 8
MASKV = -960.0  # additive pre-scale mask; -30 after the 1/sqrt(D) scale

_compiled_nc = None
last_result = None  # BassKernelResults of the most recent run (for test.py)


def _trace_kernel(tc, out, xT, xqT, wqT, wkT, wvT, maskadd):
    nc = tc.nc
    f32 = mybir.dt.float32
    bf16 = mybir.dt.bfloat16
    ts = bass.ts

    with (
        tc.tile_pool(name="sb", bufs=1) as sb,
        tc.tile_pool(name="ps", bufs=2, space="PSUM") as ps,
    ):
        # ---- persistent SBUF ----
        xT_s = sb.tile([P, EC, S], bf16)    # x[b]^T  (e on partitions)
        xqT_s = sb.tile([P, EC, D], bf16)   # own-query columns of x^T
        KT_s = sb.tile([P, EC, S], bf16)    # K^T (d on partitions)
        V_s = sb.tile([P, NKB, D], bf16)    # V natural (s on partitions)
        QT_s = sb.tile([P, EC, D], bf16)    # Q^T for own queries
        mask_s = sb.tile([P, NQB, 2 * P], f32)
        ident = sb.tile([P, P], bf16)
        make_identity(nc, ident)

        for ec in range(EC):
            nc.sync.dma_start(xqT_s[:, ec], xqT[ts(ec, P), :])
        for j in range(NQB):
            nc.sync.dma_start(mask_s[:, j], maskadd[j])
        for ec in range(EC):
            nc.sync.dma_start(xT_s[:, ec], xT[ts(ec, P), :])

        def load_w(w_dram, nm):
            w_s = sb.tile([P, EC, D], bf16, tag="w", bufs=2, name=nm)
            for ec in range(EC):
                nc.sync.dma_start(w_s[:, ec], w_dram[ts(ec, P), :])
            return w_s

        # ---- Q^T projection: QT[d, q] = sum_e WqT[e, d] * xqT[e, q] ----
        wq_s = load_w(wqT, "wq_s")
        for dc in range(EC):
            acc = ps.tile([P, D], f32, tag="big")
            for ec in range(EC):
                lhsT = wq_s[:, ec, ts(dc, P)]
                for nh in range(2):
                    nc.tensor.matmul(
                        acc[:, ts(nh, 512)], lhsT, xqT_s[:, ec, ts(nh, 512)],
                        start=(ec == 0), stop=(ec == EC - 1))
            nc.scalar.copy(QT_s[:, dc], acc)

        # ---- K^T projection, s-chunk-major so early key columns finish first ----
        wk_s = load_w(wkT, "wk_s")
        for sc in range(S // 512):
            for dc in range(EC):
                acc = ps.tile([P, 512], f32, tag="s")
                for ec in range(EC):
                    nc.tensor.matmul(
                        acc, wk_s[:, ec, ts(dc, P)], xT_s[:, ec, ts(sc, 512)],
                        start=(ec == 0), stop=(ec == EC - 1))
                nc.scalar.copy(KT_s[:, dc, ts(sc, 512)], acc)

        # ---- V projection: V[s, d] = sum_e xT[e, s] * WvT[e, d] ----
        wv_s = load_w(wvT, "wv_s")
        for kb in range(NKB):
            acc = ps.tile([P, D], f32, tag="big")
            for ec in range(EC):
                lhsT = xT_s[:, ec, ts(kb, P)]
                for nh in range(2):
                    nc.tensor.matmul(
                        acc[:, ts(nh, 512)], lhsT, wv_s[:, ec, ts(nh, 512)],
                        start=(ec == 0), stop=(ec == EC - 1))
            nc.vector.tensor_copy(V_s[:, kb], acc)

        # ---- attention, one 128-row query block at a time ----
        inv_sqrt_d = 1.0 / float(np.sqrt(D))
        for j in range(NQB):
            nkt = 2 * j + 2          # key tiles (uniform across cores)
            ncols = nkt * P
            nch = (ncols + 511) // 512
            p_sb = sb.tile([P, S], bf16, tag="p_sb", bufs=2)
            dsl = sb.tile([P, 4], f32, tag="dsl", bufs=2)
            for ch in range(nch):
                c0 = ch * 512
                cw = min(512, ncols - c0)
                sfull = ps.tile([P, 512], f32, tag="s")
                sps = sfull[:, :cw]
                for dc in range(EC):
                    nc.tensor.matmul(
                        sps, QT_s[:, dc, ts(j, P)], KT_s[:, dc, c0:c0 + cw],
                        start=(dc == 0), stop=(dc == EC - 1))
                if c0 + cw == ncols:  # last chunk holds the 2 maskable tiles
                    nc.vector.tensor_add(
                        sps[:, cw - 2 * P:cw], sps[:, cw - 2 * P:cw], mask_s[:, j])
                nc.scalar.activation(
                    p_sb[:, c0:c0 + cw], sps,
                    mybir.ActivationFunctionType.Exp,
                    scale=inv_sqrt_d,
                    accum_out=dsl[:, ch:ch + 1])

            denom = sb.tile([P, 1], f32, tag="den", bufs=2)
            nc.vector.reduce_sum(denom, dsl[:, :nch], axis=mybir.AxisListType.X)
            recip = sb.tile([P, 1], f32, tag="rcp", bufs=2)
            nc.vector.reciprocal(recip, denom)

            pts = []
            for kt in range(nkt):
                ptp = ps.tile([P, P], bf16, tag="pt")
                nc.tensor.transpose(ptp, p_sb[:, ts(kt, P)], ident)
                pt_sb = sb.tile([P, P], bf16, tag="pt_sb", bufs=16)
                nc.vector.tensor_copy(pt_sb, ptp)
                pts.append(pt_sb)

            acc = ps.tile([P, D], f32, tag="big")
            for kt in range(nkt):
                for nh in range(2):
                    nc.tensor.matmul(
                        acc[:, ts(nh, 512)], pts[kt], V_s[:, kt, ts(nh, 512)],
                        start=(kt == 0), stop=(kt == nkt - 1))
            o_sb = sb.tile([P, D], f32, tag="o_sb", bufs=2)
            nc.vector.tensor_scalar_mul(o_sb, acc, recip)
            nc.sync.dma_start(out[j], o_sb)


def build_nc(debug=False):
    nc = bacc.Bacc("TRN2", target_bir_lowering=False, debug=debug,
                   enable_asserts=False, num_devices=NCORES)
    bf16 = mybir.dt.bfloat16
    f32 = mybir.dt.float32
    xT = nc.dram_tensor("xT", (D, S), bf16, kind="ExternalInput").ap()
    xqT = nc.dram_tensor("xqT", (D, D), bf16, kind="ExternalInput").ap()
    wqT = nc.dram_tensor("wqT", (D, D), bf16, kind="ExternalInput").ap()
    wkT = nc.dram_tensor("wkT", (D, D), bf16, kind="ExternalInput").ap()
    wvT = nc.dram_tensor("wvT", (D, D), bf16, kind="ExternalInput").ap()
    maskadd = nc.dram_tensor("maskadd", (NQB, P, 2 * P), f32,
                             kind="ExternalInput").ap()
    out = nc.dram_tensor("out", (NQB, P, D), f32, kind="ExternalOutput").ap()
    with tile.TileContext(nc) as tc:
        _trace_kernel(tc, out, xT, xqT, wqT, wkT, wvT, maskadd)
    nc.compile()
    return nc


def _get_compiled():
    global _compiled_nc
    if _compiled_nc is None:
        _compiled_nc = build_nc(debug=False)
    return _compiled_nc


def make_in_maps(x):
    """Per-core host-side slicing + layout prep (no matmul math here)."""
    x = np.asarray(x, dtype=np.float32)
    r = np.arange(P)
    tri_add = np.where(r[None, :] <= r[:, None], 0.0, MASKV).astype(np.float32)
    mask_h = []
    for h in range(2):
        if h == 0:
            blk = np.concatenate(
                [tri_add, np.full((P, P), MASKV, np.float32)], axis=1)
        else:
            blk = np.concatenate([np.zeros((P, P), np.float32), tri_add], axis=1)
        mask_h.append(np.ascontiguousarray(
            np.broadcast_to(blk, (NQB, P, 2 * P))).astype(np.float32))

    in_maps = []
    xT_b = {}
    for c in range(NCORES):
        b, h = c // 2, c % 2
        if b not in xT_b:
            xT_b[b] = np.ascontiguousarray(x[b].T).astype(BF16)
        blocks = [2 * j + h for j in range(NQB)]
        xq = np.concatenate([x[b][g * P:(g + 1) * P] for g in blocks], axis=0)
        xqT = np.ascontiguousarray(xq.T).astype(BF16)
        in_maps.append({
            "xT": xT_b[b],
            "xqT": xqT,
            "maskadd": mask_h[h],
        })
    return in_maps


def kernel(x, Wq, bq, Wk, bk, Wv, bv, mask):
    global last_result
    x = np.asarray(x, np.float32)
    Wq = np.asarray(Wq, np.float32)
    Wk = np.asarray(Wk, np.float32)
    Wv = np.asarray(Wv, np.float32)
    bq = np.asarray(bq, np.float32)
    bk = np.asarray(bk, np.float32)
    bv = np.asarray(bv, np.float32)
    mask = np.asarray(mask)

    causal = bool(np.array_equal(mask != 0, np.tril(np.ones(mask.shape, bool))))
    if np.any(bq) or np.any(bk) or not causal:
        return _np_reference(x, Wq, bq, Wk, bk, Wv, bv, mask)

    nc = _get_compiled()
    in_maps = make_in_maps(x)
    wT = {
        "wqT": np.ascontiguousarray(Wq.T).astype(BF16),
        "wkT": np.ascontiguousarray(Wk.T).astype(BF16),
        "wvT": np.ascontiguousarray(Wv.T).astype(BF16),
    }
    for m in in_maps:
        m.update(wT)

    res = bass_utils.run_bass_kernel_spmd(nc, in_maps, core_ids=list(range(NCORES)))
    last_result = res

    out = np.empty((B * S, D), np.float32)
    for c in range(NCORES):
        b, h = c // 2, c % 2
        o = np.asarray(res.results[c]["out"], np.float32)
        for j in range(NQB):
            g = 2 * j + h
            out[b * S + g * P: b * S + (g + 1) * P] = o[j]
    if np.any(bv):
        out = out + bv[None, :]  # attn rows sum to 1, so bv adds exactly
    return out


def _np_reference(x, Wq, bq, Wk, bk, Wv, bv, mask):
    outs = []
    for b in range(x.shape[0]):
        xb = x[b]
        Q = xb @ Wq.T + bq
        K = xb @ Wk.T + bk
        V = xb @ Wv.T + bv
        Sc = (Q @ K.T) / np.float32(np.sqrt(x.shape[2]))
        Sc = np.where(mask == 0, np.float32(-1e9), Sc)
        Sc = Sc - Sc.max(axis=1, keepdims=True)
        E = np.exp(Sc)
        A = E / E.sum(axis=1, keepdims=True)
        outs.append(A @ V)
    return np.concatenate(outs, axis=0).astype(np.float32)


# revision 8
# speedup vs baseline: 511.0470x; 511.0470x over previous
"""Causal self-attention (B=4, S=2048, D=1024, single 1024-wide head) on 8 TRN2 cores.

Sharding: core c -> batch b=c//2, parity h=c%2. Each core computes K/V for its
whole batch (duplicated across the 2 cores of a batch) and handles the 8
query blocks {h, h+2, ..., h+14} (128 rows each). Pairing strided blocks keeps
causal work balanced and — with key-extents padded to 256*(j+1) — makes the
program identical on every core; causality differences live in per-core
additive-mask input data, not control flow.

All matmuls run on the PE in bf16 with fp32 PSUM accumulation. Softmax skips
max-subtraction (scores are ~N(0,1); exp stays in fp32 range) so the
denominator comes free from the Exp activation's accumulate output.
"""

import numpy as np
import ml_dtypes

import concourse.bass as bass
import concourse.bacc as bacc
import concourse.tile as tile
from concourse import mybir
from concourse import bass_utils
from concourse.masks import make_identity

BF16 = ml_dtypes.bfloat16
P = 128
B, S, D = 4, 2048, 1024
EC = D // P  # contraction chunks (8)
NQB = 8      # query blocks per core
NKB = S // P  # key blocks per batch (16)
NCORES = 8
MASKV = -960.0  # additive pre-scale mask; -30 after the 1/sqrt(D) scale

_compiled_nc = None
_runner = None  # cached (sharded_jit, in_names, out_names, out_avals, n_params)
last_result = None  # kept for compatibility with older test harnesses


def _trace_kernel(tc, out, xT, xqT, wqT, wkT, wvT, maskadd):
    nc = tc.nc
    f32 = mybir.dt.float32
    bf16 = mybir.dt.bfloat16
    ts = bass.ts

    with (
        tc.tile_pool(name="sb", bufs=1) as sb,
        tc.tile_pool(name="ps", bufs=2, space="PSUM") as ps,
    ):
        # ---- persistent SBUF ----
        xT_s = sb.tile([P, EC, S], bf16)    # x[b]^T  (e on partitions)
        xqT_s = sb.tile([P, EC, D], bf16)   # own-query columns of x^T
        KT_s = sb.tile([P, EC, S], bf16)    # K^T (d on partitions)
        V_s = sb.tile([P, NKB, D], bf16)    # V natural (s on partitions)
        QT_s = sb.tile([P, EC, D], bf16)    # Q^T for own queries
        mask_s = sb.tile([P, NQB, 2 * P], f32)
        ident = sb.tile([P, P], bf16)
        make_identity(nc, ident)

        def load_w(w_dram, nm):
            w_s = sb.tile([P, EC, D], bf16, tag="w", bufs=2, name=nm)
            for ec in range(EC):
                nc.sync.dma_start(w_s[:, ec], w_dram[ts(ec, P), :])
            return w_s

        # interleave the first projection's operands so PE starts ASAP
        wq_s = sb.tile([P, EC, D], bf16, tag="w", bufs=2, name="wq_s")
        for ec in range(EC):
            nc.sync.dma_start(wq_s[:, ec], wqT[ts(ec, P), :])
            nc.sync.dma_start(xqT_s[:, ec], xqT[ts(ec, P), :])
        for ec in range(EC):
            nc.sync.dma_start(xT_s[:, ec], xT[ts(ec, P), :])
        for j in range(NQB):
            nc.sync.dma_start(mask_s[:, j], maskadd[j])

        # ---- Q^T projection: QT[d, q] = sum_e WqT[e, d] * xqT[e, q] ----
        for dc in range(EC):
            acc = ps.tile([P, D], f32, tag="big")
            for ec in range(EC):
                lhsT = wq_s[:, ec, ts(dc, P)]
                for nh in range(2):
                    nc.tensor.matmul(
                        acc[:, ts(nh, 512)], lhsT, xqT_s[:, ec, ts(nh, 512)],
                        start=(ec == 0), stop=(ec == EC - 1))
            nc.scalar.copy(QT_s[:, dc], acc)

        # ---- K^T projection, s-chunk-major so early key columns finish first ----
        wk_s = load_w(wkT, "wk_s")
        for sc in range(S // 512):
            for dc in range(EC):
                acc = ps.tile([P, 512], f32, tag="s")
                for ec in range(EC):
                    nc.tensor.matmul(
                        acc, wk_s[:, ec, ts(dc, P)], xT_s[:, ec, ts(sc, 512)],
                        start=(ec == 0), stop=(ec == EC - 1))
                nc.scalar.copy(KT_s[:, dc, ts(sc, 512)], acc)

        # ---- V projection: V[s, d] = sum_e xT[e, s] * WvT[e, d] ----
        wv_s = load_w(wvT, "wv_s")
        for kb in range(NKB):
            acc = ps.tile([P, D], f32, tag="big")
            for ec in range(EC):
                lhsT = xT_s[:, ec, ts(kb, P)]
                for nh in range(2):
                    nc.tensor.matmul(
                        acc[:, ts(nh, 512)], lhsT, wv_s[:, ec, ts(nh, 512)],
                        start=(ec == 0), stop=(ec == EC - 1))
            nc.vector.tensor_copy(V_s[:, kb], acc)

        # ---- attention, one 128-row query block at a time ----
        inv_sqrt_d = 1.0 / float(np.sqrt(D))
        for j in reversed(range(NQB)):  # big blocks first: shortest tail ends the kernel
            nkt = 2 * j + 2          # key tiles (uniform across cores)
            ncols = nkt * P
            nch = (ncols + 511) // 512
            p_sb = sb.tile([P, S], bf16, tag="p_sb", bufs=2)
            dsl = sb.tile([P, 4], f32, tag="dsl", bufs=2)
            for ch in range(nch):
                c0 = ch * 512
                cw = min(512, ncols - c0)
                sfull = ps.tile([P, 512], f32, tag="s")
                sps = sfull[:, :cw]
                for dc in range(EC):
                    nc.tensor.matmul(
                        sps, QT_s[:, dc, ts(j, P)], KT_s[:, dc, c0:c0 + cw],
                        start=(dc == 0), stop=(dc == EC - 1))
                if c0 + cw == ncols:  # last chunk holds the 2 maskable tiles
                    nc.vector.tensor_add(
                        sps[:, cw - 2 * P:cw], sps[:, cw - 2 * P:cw], mask_s[:, j])
                nc.scalar.activation(
                    p_sb[:, c0:c0 + cw], sps,
                    mybir.ActivationFunctionType.Exp,
                    scale=inv_sqrt_d,
                    accum_out=dsl[:, ch:ch + 1])

            denom = sb.tile([P, 1], f32, tag="den", bufs=2)
            nc.vector.reduce_sum(denom, dsl[:, :nch], axis=mybir.AxisListType.X)
            recip = sb.tile([P, 1], f32, tag="rcp", bufs=2)
            nc.vector.reciprocal(recip, denom)

            pts = []
            for kt in range(nkt):
                ptp = ps.tile([P, P], bf16, tag="pt")
                nc.tensor.transpose(ptp, p_sb[:, ts(kt, P)], ident)
                pt_sb = sb.tile([P, P], bf16, tag="pt_sb", bufs=16)
                nc.vector.tensor_copy(pt_sb, ptp)
                pts.append(pt_sb)

            acc = ps.tile([P, D], f32, tag="big")
            for kt in range(nkt):
                for nh in range(2):
                    nc.tensor.matmul(
                        acc[:, ts(nh, 512)], pts[kt], V_s[:, kt, ts(nh, 512)],
                        start=(kt == 0), stop=(kt == nkt - 1))
            o_sb = sb.tile([P, D], f32, tag="o_sb", bufs=2)
            # normalize on ACT (idle here) so DVE stays free for PT copies
            nc.scalar.activation(o_sb, acc, mybir.ActivationFunctionType.Copy,
                                 scale=recip)
            nc.sync.dma_start(out[j], o_sb)


def build_nc(debug=False):
    nc = bacc.Bacc("TRN2", target_bir_lowering=False, debug=debug,
                   enable_asserts=False, num_devices=NCORES)
    bf16 = mybir.dt.bfloat16
    f32 = mybir.dt.float32
    xT = nc.dram_tensor("xT", (D, S), bf16, kind="ExternalInput").ap()
    xqT = nc.dram_tensor("xqT", (D, D), bf16, kind="ExternalInput").ap()
    wqT = nc.dram_tensor("wqT", (D, D), bf16, kind="ExternalInput").ap()
    wkT = nc.dram_tensor("wkT", (D, D), bf16, kind="ExternalInput").ap()
    wvT = nc.dram_tensor("wvT", (D, D), bf16, kind="ExternalInput").ap()
    maskadd = nc.dram_tensor("maskadd", (NQB, P, 2 * P), f32,
                             kind="ExternalInput").ap()
    out = nc.dram_tensor("out", (NQB, P, D), f32, kind="ExternalOutput").ap()
    with tile.TileContext(nc) as tc:
        _trace_kernel(tc, out, xT, xqT, wqT, wkT, wvT, maskadd)
    nc.compile()
    return nc


def _get_compiled():
    global _compiled_nc
    if _compiled_nc is None:
        _compiled_nc = build_nc(debug=False)
    return _compiled_nc


def _get_runner():
    """Jit-once shard_map runner over the 8 NeuronCores.

    Mirrors bass2jax.run_bass_via_pjrt's multi-core branch, but caches the
    jitted executable so repeat kernel() calls skip retracing/recompiling.
    """
    global _runner
    if _runner is not None:
        return _runner
    import jax
    from jax.experimental.shard_map import shard_map
    from jax.sharding import Mesh, PartitionSpec
    from concourse import bass2jax

    nc = _get_compiled()
    bass2jax.install_neuronx_cc_hook()

    partition_name = (nc.partition_id_tensor.name
                      if nc.partition_id_tensor else None)
    in_names, out_names, out_avals, zero_outs = [], [], [], []
    for alloc in nc.m.functions[0].allocations:
        if not isinstance(alloc, mybir.MemoryLocationSet):
            continue
        name = alloc.memorylocations[0].name
        if alloc.kind == "ExternalInput":
            if name != partition_name:
                in_names.append(name)
        elif alloc.kind == "ExternalOutput":
            shape = tuple(alloc.tensor_shape)
            dtype = mybir.dt.np(alloc.dtype)
            out_names.append(name)
            out_avals.append(jax.core.ShapedArray(shape, dtype))
            zero_outs.append(np.zeros(shape, dtype))
    n_params = len(in_names)
    all_in_names = list(in_names) + list(out_names)
    if partition_name is not None:
        all_in_names.append(partition_name)
    donate = tuple(range(n_params, n_params + len(out_names)))

    def _body(*args):
        operands = list(args)
        if partition_name is not None:
            operands.append(bass2jax.partition_id_tensor())
        outs = bass2jax._bass_exec_p.bind(
            *operands,
            out_avals=tuple(out_avals),
            in_names=tuple(all_in_names),
            out_names=tuple(out_names),
            lowering_input_output_aliases=(),
            sim_require_finite=True,
            sim_require_nnan=True,
            nc=nc,
        )
        return tuple(outs)

    devices = jax.devices()[:NCORES]
    mesh = Mesh(np.asarray(devices), ("core",))
    nin = n_params + len(out_names)
    sharded = jax.jit(
        shard_map(_body, mesh=mesh,
                  in_specs=(PartitionSpec("core"),) * nin,
                  out_specs=(PartitionSpec("core"),) * len(out_names),
                  check_rep=False),
        donate_argnums=donate, keep_unused=True)
    _runner = (sharded, in_names, out_names, out_avals, n_params, zero_outs, mesh)
    return _runner


def run_device(in_maps):
    """Execute the compiled NEFF on all 8 cores; returns per-core output dicts."""
    sharded, in_names, out_names, out_avals, n_params, zero_outs, _ = _get_runner()
    concat_in = [
        np.concatenate([np.asarray(in_maps[c][nm]) for c in range(NCORES)], axis=0)
        for nm in in_names
    ]
    concat_zeros = [
        np.zeros((NCORES * z.shape[0], *z.shape[1:]), z.dtype) for z in zero_outs
    ]
    out_arrs = sharded(*concat_in, *concat_zeros)
    return [
        {nm: np.asarray(out_arrs[i]).reshape(NCORES, *out_avals[i].shape)[c]
         for i, nm in enumerate(out_names)}
        for c in range(NCORES)
    ]


def make_in_maps(x):
    """Per-core host-side slicing + layout prep (no matmul math here)."""
    x = np.asarray(x, dtype=np.float32)
    r = np.arange(P)
    tri_add = np.where(r[None, :] <= r[:, None], 0.0, MASKV).astype(np.float32)
    mask_h = []
    for h in range(2):
        if h == 0:
            blk = np.concatenate(
                [tri_add, np.full((P, P), MASKV, np.float32)], axis=1)
        else:
            blk = np.concatenate([np.zeros((P, P), np.float32), tri_add], axis=1)
        mask_h.append(np.ascontiguousarray(
            np.broadcast_to(blk, (NQB, P, 2 * P))).astype(np.float32))

    in_maps = []
    xT_b = {}
    for c in range(NCORES):
        b, h = c // 2, c % 2
        if b not in xT_b:
            xT_b[b] = np.ascontiguousarray(x[b].T).astype(BF16)
        blocks = [2 * j + h for j in range(NQB)]
        xq = np.concatenate([x[b][g * P:(g + 1) * P] for g in blocks], axis=0)
        xqT = np.ascontiguousarray(xq.T).astype(BF16)
        in_maps.append({
            "xT": xT_b[b],
            "xqT": xqT,
            "maskadd": mask_h[h],
        })
    return in_maps


def kernel(x, Wq, bq, Wk, bk, Wv, bv, mask):
    global last_result
    x = np.asarray(x, np.float32)
    Wq = np.asarray(Wq, np.float32)
    Wk = np.asarray(Wk, np.float32)
    Wv = np.asarray(Wv, np.float32)
    bq = np.asarray(bq, np.float32)
    bk = np.asarray(bk, np.float32)
    bv = np.asarray(bv, np.float32)
    mask = np.asarray(mask)

    causal = bool(np.array_equal(mask != 0, np.tril(np.ones(mask.shape, bool))))
    if np.any(bq) or np.any(bk) or not causal:
        return _np_reference(x, Wq, bq, Wk, bk, Wv, bv, mask)

    in_maps = make_in_maps(x)
    wT = {
        "wqT": np.ascontiguousarray(Wq.T).astype(BF16),
        "wkT": np.ascontiguousarray(Wk.T).astype(BF16),
        "wvT": np.ascontiguousarray(Wv.T).astype(BF16),
    }
    for m in in_maps:
        m.update(wT)

    results = run_device(in_maps)

    out = np.empty((B * S, D), np.float32)
    for c in range(NCORES):
        b, h = c // 2, c % 2
        o = np.asarray(results[c]["out"], np.float32)
        for j in range(NQB):
            g = 2 * j + h
            out[b * S + g * P: b * S + (g + 1) * P] = o[j]
    if np.any(bv):
        out = out + bv[None, :]  # attn rows sum to 1, so bv adds exactly
    return out


def _np_reference(x, Wq, bq, Wk, bk, Wv, bv, mask):
    outs = []
    for b in range(x.shape[0]):
        xb = x[b]
        Q = xb @ Wq.T + bq
        K = xb @ Wk.T + bk
        V = xb @ Wv.T + bv
        Sc = (Q @ K.T) / np.float32(np.sqrt(x.shape[2]))
        Sc = np.where(mask == 0, np.float32(-1e9), Sc)
        Sc = Sc - Sc.max(axis=1, keepdims=True)
        E = np.exp(Sc)
        A = E / E.sum(axis=1, keepdims=True)
        outs.append(A @ V)
    return np.concatenate(outs, axis=0).astype(np.float32)


# revision 12
# speedup vs baseline: 531.0713x; 1.0392x over previous
"""Causal self-attention (B=4, S=2048, D=1024, single 1024-wide head) on 8 TRN2 cores.

Sharding: core c -> batch b=c//2, parity h=c%2. Each core computes K/V for its
whole batch (duplicated across the 2 cores of a batch) and handles the 8
query blocks {h, h+2, ..., h+14} (128 rows each). Pairing strided blocks keeps
causal work balanced and — with key-extents padded to 256*(j+1) — makes the
program identical on every core; causality differences live in per-core
additive-mask input data, not control flow.

All matmuls run on the PE in bf16 with fp32 PSUM accumulation. Softmax skips
max-subtraction (scores are ~N(0,1); exp stays in fp32 range) so the
denominator comes free from the Exp activation's accumulate output.
"""

import numpy as np
import ml_dtypes

import concourse.bass as bass
import concourse.bacc as bacc
import concourse.tile as tile
from concourse import mybir
from concourse import bass_utils
from concourse.masks import make_identity

BF16 = ml_dtypes.bfloat16
P = 128
B, S, D = 4, 2048, 1024
EC = D // P  # contraction chunks (8)
NQB = 8      # query blocks per core
NKB = S // P  # key blocks per batch (16)
NCORES = 8
MASKV = -960.0  # additive pre-scale mask; -30 after the 1/sqrt(D) scale

_compiled_nc = None
_runner = None  # cached (sharded_jit, in_names, out_names, out_avals, n_params)
last_result = None  # kept for compatibility with older test harnesses


def _trace_kernel(tc, out, xT, xqT, wqT, wkT, wvT, maskadd):
    nc = tc.nc
    f32 = mybir.dt.float32
    bf16 = mybir.dt.bfloat16
    ts = bass.ts

    with (
        tc.tile_pool(name="sb", bufs=1) as sb,
        tc.tile_pool(name="ps", bufs=2, space="PSUM") as ps,
        tc.tile_pool(name="dram", bufs=1, space="DRAM") as dram,
    ):
        # ---- persistent SBUF ----
        xT_s = sb.tile([P, EC, S // 2], bf16)  # OWN HALF of x[b]^T (e on parts)
        xqT_s = sb.tile([P, EC, D], bf16)   # own-query columns of x^T
        KT_s = sb.tile([P, EC, S], bf16)    # K^T (d on partitions), both halves
        V_s = sb.tile([P, NKB, D], bf16)    # V natural (s on partitions)
        QT_s = sb.tile([P, EC, D], bf16)    # Q^T for own queries
        mask_s = sb.tile([P, NQB, 2 * P], f32)
        ident = sb.tile([P, P], bf16)
        make_identity(nc, ident)

        # DRAM bounce buffers for the pair AllGather of the K^T / V halves
        kin = dram.tile([EC, P, S // 2], bf16)
        kout = dram.tile([2, EC, P, S // 2], bf16)
        vin = dram.tile([NKB // 2, P, D], bf16)
        vout = dram.tile([2, NKB // 2, P, D], bf16)
        groups = [[2 * b, 2 * b + 1] for b in range(NCORES // 2)]

        def load_w(w_dram, nm):
            w_s = sb.tile([P, EC, D], bf16, tag="w", bufs=2, name=nm)
            for ec in range(EC):
                nc.sync.dma_start(w_s[:, ec], w_dram[ts(ec, P), :])
            return w_s

        # interleave the first projection's operands so PE starts ASAP
        wq_s = sb.tile([P, EC, D], bf16, tag="w", bufs=2, name="wq_s")
        for ec in range(EC):
            nc.sync.dma_start(wq_s[:, ec], wqT[ts(ec, P), :])
            nc.sync.dma_start(xqT_s[:, ec], xqT[ts(ec, P), :])
        for ec in range(EC):
            nc.sync.dma_start(xT_s[:, ec], xT[ts(ec, P), :])
        for j in range(NQB):
            nc.sync.dma_start(mask_s[:, j], maskadd[j])

        # ---- Q^T projection: QT[d, q] = sum_e WqT[e, d] * xqT[e, q] ----
        for dc in range(EC):
            acc = ps.tile([P, D], f32, tag="big")
            for ec in range(EC):
                lhsT = wq_s[:, ec, ts(dc, P)]
                for nh in range(2):
                    nc.tensor.matmul(
                        acc[:, ts(nh, 512)], lhsT, xqT_s[:, ec, ts(nh, 512)],
                        start=(ec == 0), stop=(ec == EC - 1))
            nc.scalar.copy(QT_s[:, dc], acc)

        # ---- K^T projection for the OWN sequence half only ----
        wk_s = load_w(wkT, "wk_s")
        for sc in range(S // 2 // 512):
            for dc in range(EC):
                acc = ps.tile([P, 512], f32, tag="s")
                for ec in range(EC):
                    nc.tensor.matmul(
                        acc, wk_s[:, ec, ts(dc, P)], xT_s[:, ec, ts(sc, 512)],
                        start=(ec == 0), stop=(ec == EC - 1))
                kstg = sb.tile([P, 512], bf16, tag="kstg", bufs=4)
                nc.scalar.copy(kstg, acc)
                nc.sync.dma_start(kin[dc, :, ts(sc, 512)], kstg)

        # pair AllGather: rank order [even, odd] = [first half, second half]
        nc.gpsimd.collective_compute(
            "AllGather", mybir.AluOpType.bypass,
            replica_groups=groups, ins=[kin.opt()], outs=[kout.opt()])
        for r in range(2):
            for dc in range(EC):
                nc.sync.dma_start(
                    KT_s[:, dc, r * (S // 2):(r + 1) * (S // 2)], kout[r, dc])

        # ---- V projection for the OWN half: V[s, d] ----
        wv_s = load_w(wvT, "wv_s")
        for kb in range(NKB // 2):
            acc = ps.tile([P, D], f32, tag="big")
            for ec in range(EC):
                lhsT = xT_s[:, ec, ts(kb, P)]
                for nh in range(2):
                    nc.tensor.matmul(
                        acc[:, ts(nh, 512)], lhsT, wv_s[:, ec, ts(nh, 512)],
                        start=(ec == 0), stop=(ec == EC - 1))
            vstg = sb.tile([P, D], bf16, tag="vstg", bufs=4)
            nc.vector.tensor_copy(vstg, acc)
            nc.sync.dma_start(vin[kb], vstg)

        nc.gpsimd.collective_compute(
            "AllGather", mybir.AluOpType.bypass,
            replica_groups=groups, ins=[vin.opt()], outs=[vout.opt()])
        for r in range(2):
            for kb in range(NKB // 2):
                nc.sync.dma_start(V_s[:, r * (NKB // 2) + kb, :], vout[r, kb])

        # ---- attention, one 128-row query block at a time ----
        inv_sqrt_d = 1.0 / float(np.sqrt(D))
        for j in reversed(range(NQB)):  # big blocks first: shortest tail ends the kernel
            nkt = 2 * j + 2          # key tiles (uniform across cores)
            ncols = nkt * P
            nch = (ncols + 511) // 512
            p_sb = sb.tile([P, S], bf16, tag="p_sb", bufs=2)
            dsl = sb.tile([P, 4], f32, tag="dsl", bufs=2)
            for ch in range(nch):
                c0 = ch * 512
                cw = min(512, ncols - c0)
                sfull = ps.tile([P, 512], f32, tag="s")
                sps = sfull[:, :cw]
                for dc in range(EC):
                    nc.tensor.matmul(
                        sps, QT_s[:, dc, ts(j, P)], KT_s[:, dc, c0:c0 + cw],
                        start=(dc == 0), stop=(dc == EC - 1))
                if c0 + cw == ncols:  # last chunk holds the 2 maskable tiles
                    nc.vector.tensor_add(
                        sps[:, cw - 2 * P:cw], sps[:, cw - 2 * P:cw], mask_s[:, j])
                nc.scalar.activation(
                    p_sb[:, c0:c0 + cw], sps,
                    mybir.ActivationFunctionType.Exp,
                    scale=inv_sqrt_d,
                    accum_out=dsl[:, ch:ch + 1])

            denom = sb.tile([P, 1], f32, tag="den", bufs=2)
            nc.vector.reduce_sum(denom, dsl[:, :nch], axis=mybir.AxisListType.X)
            recip = sb.tile([P, 1], f32, tag="rcp", bufs=2)
            nc.vector.reciprocal(recip, denom)

            pts = []
            for kt in range(nkt):
                ptp = ps.tile([P, P], bf16, tag="pt")
                nc.tensor.transpose(ptp, p_sb[:, ts(kt, P)], ident)
                pt_sb = sb.tile([P, P], bf16, tag="pt_sb", bufs=16)
                nc.vector.tensor_copy(pt_sb, ptp)
                pts.append(pt_sb)

            acc = ps.tile([P, D], f32, tag="big")
            for kt in range(nkt):
                for nh in range(2):
                    nc.tensor.matmul(
                        acc[:, ts(nh, 512)], pts[kt], V_s[:, kt, ts(nh, 512)],
                        start=(kt == 0), stop=(kt == nkt - 1))
            o_sb = sb.tile([P, D], f32, tag="o_sb", bufs=2)
            # normalize on ACT (idle here) so DVE stays free for PT copies
            nc.scalar.activation(o_sb, acc, mybir.ActivationFunctionType.Copy,
                                 scale=recip)
            nc.sync.dma_start(out[j], o_sb)


def build_nc(debug=False):
    nc = bacc.Bacc("TRN2", target_bir_lowering=False, debug=debug,
                   enable_asserts=False, num_devices=NCORES)
    bf16 = mybir.dt.bfloat16
    f32 = mybir.dt.float32
    xT = nc.dram_tensor("xT", (D, S // 2), bf16, kind="ExternalInput").ap()
    xqT = nc.dram_tensor("xqT", (D, D), bf16, kind="ExternalInput").ap()
    wqT = nc.dram_tensor("wqT", (D, D), bf16, kind="ExternalInput").ap()
    wkT = nc.dram_tensor("wkT", (D, D), bf16, kind="ExternalInput").ap()
    wvT = nc.dram_tensor("wvT", (D, D), bf16, kind="ExternalInput").ap()
    maskadd = nc.dram_tensor("maskadd", (NQB, P, 2 * P), f32,
                             kind="ExternalInput").ap()
    out = nc.dram_tensor("out", (NQB, P, D), f32, kind="ExternalOutput").ap()
    with tile.TileContext(nc) as tc:
        _trace_kernel(tc, out, xT, xqT, wqT, wkT, wvT, maskadd)
    nc.compile()
    return nc


def _get_compiled():
    global _compiled_nc
    if _compiled_nc is None:
        _compiled_nc = build_nc(debug=False)
    return _compiled_nc


def _get_runner():
    """Jit-once shard_map runner over the 8 NeuronCores.

    Mirrors bass2jax.run_bass_via_pjrt's multi-core branch, but caches the
    jitted executable so repeat kernel() calls skip retracing/recompiling.
    """
    global _runner
    if _runner is not None:
        return _runner
    import jax
    from jax.experimental.shard_map import shard_map
    from jax.sharding import Mesh, PartitionSpec
    from concourse import bass2jax

    nc = _get_compiled()
    bass2jax.install_neuronx_cc_hook()

    partition_name = (nc.partition_id_tensor.name
                      if nc.partition_id_tensor else None)
    in_names, out_names, out_avals, zero_outs = [], [], [], []
    for alloc in nc.m.functions[0].allocations:
        if not isinstance(alloc, mybir.MemoryLocationSet):
            continue
        name = alloc.memorylocations[0].name
        if alloc.kind == "ExternalInput":
            if name != partition_name:
                in_names.append(name)
        elif alloc.kind == "ExternalOutput":
            shape = tuple(alloc.tensor_shape)
            dtype = mybir.dt.np(alloc.dtype)
            out_names.append(name)
            out_avals.append(jax.core.ShapedArray(shape, dtype))
            zero_outs.append(np.zeros(shape, dtype))
    n_params = len(in_names)
    all_in_names = list(in_names) + list(out_names)
    if partition_name is not None:
        all_in_names.append(partition_name)
    donate = tuple(range(n_params, n_params + len(out_names)))

    def _body(*args):
        operands = list(args)
        if partition_name is not None:
            operands.append(bass2jax.partition_id_tensor())
        outs = bass2jax._bass_exec_p.bind(
            *operands,
            out_avals=tuple(out_avals),
            in_names=tuple(all_in_names),
            out_names=tuple(out_names),
            lowering_input_output_aliases=(),
            sim_require_finite=True,
            sim_require_nnan=True,
            nc=nc,
        )
        return tuple(outs)

    devices = jax.devices()[:NCORES]
    mesh = Mesh(np.asarray(devices), ("core",))
    nin = n_params + len(out_names)
    sharded = jax.jit(
        shard_map(_body, mesh=mesh,
                  in_specs=(PartitionSpec("core"),) * nin,
                  out_specs=(PartitionSpec("core"),) * len(out_names),
                  check_rep=False),
        donate_argnums=donate, keep_unused=True)
    _runner = (sharded, in_names, out_names, out_avals, n_params, zero_outs, mesh)
    return _runner


def run_device(in_maps):
    """Execute the compiled NEFF on all 8 cores; returns per-core output dicts."""
    sharded, in_names, out_names, out_avals, n_params, zero_outs, _ = _get_runner()
    concat_in = [
        np.concatenate([np.asarray(in_maps[c][nm]) for c in range(NCORES)], axis=0)
        for nm in in_names
    ]
    concat_zeros = [
        np.zeros((NCORES * z.shape[0], *z.shape[1:]), z.dtype) for z in zero_outs
    ]
    out_arrs = sharded(*concat_in, *concat_zeros)
    return [
        {nm: np.asarray(out_arrs[i]).reshape(NCORES, *out_avals[i].shape)[c]
         for i, nm in enumerate(out_names)}
        for c in range(NCORES)
    ]


def make_in_maps(x):
    """Per-core host-side slicing + layout prep (no matmul math here)."""
    x = np.asarray(x, dtype=np.float32)
    r = np.arange(P)
    tri_add = np.where(r[None, :] <= r[:, None], 0.0, MASKV).astype(np.float32)
    mask_h = []
    for h in range(2):
        if h == 0:
            blk = np.concatenate(
                [tri_add, np.full((P, P), MASKV, np.float32)], axis=1)
        else:
            blk = np.concatenate([np.zeros((P, P), np.float32), tri_add], axis=1)
        mask_h.append(np.ascontiguousarray(
            np.broadcast_to(blk, (NQB, P, 2 * P))).astype(np.float32))

    in_maps = []
    for c in range(NCORES):
        b, h = c // 2, c % 2
        # each core owns one sequence half for the K/V projection (pair
        # AllGather exchanges them), rank order [even->first half]
        xTh = np.ascontiguousarray(x[b][h * (S // 2):(h + 1) * (S // 2)].T).astype(BF16)
        blocks = [2 * j + h for j in range(NQB)]
        xq = np.concatenate([x[b][g * P:(g + 1) * P] for g in blocks], axis=0)
        xqT = np.ascontiguousarray(xq.T).astype(BF16)
        in_maps.append({
            "xT": xTh,
            "xqT": xqT,
            "maskadd": mask_h[h],
        })
    return in_maps


def kernel(x, Wq, bq, Wk, bk, Wv, bv, mask):
    global last_result
    x = np.asarray(x, np.float32)
    Wq = np.asarray(Wq, np.float32)
    Wk = np.asarray(Wk, np.float32)
    Wv = np.asarray(Wv, np.float32)
    bq = np.asarray(bq, np.float32)
    bk = np.asarray(bk, np.float32)
    bv = np.asarray(bv, np.float32)
    mask = np.asarray(mask)

    causal = bool(np.array_equal(mask != 0, np.tril(np.ones(mask.shape, bool))))
    if np.any(bq) or np.any(bk) or not causal:
        return _np_reference(x, Wq, bq, Wk, bk, Wv, bv, mask)

    in_maps = make_in_maps(x)
    wT = {
        "wqT": np.ascontiguousarray(Wq.T).astype(BF16),
        "wkT": np.ascontiguousarray(Wk.T).astype(BF16),
        "wvT": np.ascontiguousarray(Wv.T).astype(BF16),
    }
    for m in in_maps:
        m.update(wT)

    results = run_device(in_maps)

    out = np.empty((B * S, D), np.float32)
    for c in range(NCORES):
        b, h = c // 2, c % 2
        o = np.asarray(results[c]["out"], np.float32)
        for j in range(NQB):
            g = 2 * j + h
            out[b * S + g * P: b * S + (g + 1) * P] = o[j]
    if np.any(bv):
        out = out + bv[None, :]  # attn rows sum to 1, so bv adds exactly
    return out


def _np_reference(x, Wq, bq, Wk, bk, Wv, bv, mask):
    outs = []
    for b in range(x.shape[0]):
        xb = x[b]
        Q = xb @ Wq.T + bq
        K = xb @ Wk.T + bk
        V = xb @ Wv.T + bv
        Sc = (Q @ K.T) / np.float32(np.sqrt(x.shape[2]))
        Sc = np.where(mask == 0, np.float32(-1e9), Sc)
        Sc = Sc - Sc.max(axis=1, keepdims=True)
        E = np.exp(Sc)
        A = E / E.sum(axis=1, keepdims=True)
        outs.append(A @ V)
    return np.concatenate(outs, axis=0).astype(np.float32)


# revision 16
# speedup vs baseline: 555.9058x; 1.0468x over previous
"""Causal self-attention (B=4, S=2048, D=1024, single 1024-wide head) on 8 TRN2 cores.

Sharding: core c -> batch b=c//2, parity h=c%2. Each core computes K/V for its
whole batch (duplicated across the 2 cores of a batch) and handles the 8
query blocks {h, h+2, ..., h+14} (128 rows each). Pairing strided blocks keeps
causal work balanced and — with key-extents padded to 256*(j+1) — makes the
program identical on every core; causality differences live in per-core
additive-mask input data, not control flow.

All matmuls run on the PE in bf16 with fp32 PSUM accumulation. Softmax skips
max-subtraction (scores are ~N(0,1); exp stays in fp32 range) so the
denominator comes free from the Exp activation's accumulate output.
"""

import numpy as np
import ml_dtypes

import concourse.bass as bass
import concourse.bacc as bacc
import concourse.tile as tile
from concourse import mybir
from concourse import bass_utils
from concourse.masks import make_identity

BF16 = ml_dtypes.bfloat16
P = 128
B, S, D = 4, 2048, 1024
EC = D // P  # contraction chunks (8)
NQB = 8      # query blocks per core
NKB = S // P  # key blocks per batch (16)
NCORES = 8
MASKV = -960.0  # additive pre-scale mask; -30 after the 1/sqrt(D) scale

_compiled_nc = None
_runner = None  # cached (sharded_jit, in_names, out_names, out_avals, n_params)
last_result = None  # kept for compatibility with older test harnesses


def _trace_kernel(tc, out, xT, xqT, wqT, wkT, wvT, maskadd):
    nc = tc.nc
    f32 = mybir.dt.float32
    bf16 = mybir.dt.bfloat16
    ts = bass.ts

    with (
        tc.tile_pool(name="sb", bufs=1) as sb,
        tc.tile_pool(name="ps", bufs=2, space="PSUM") as ps,
    ):
        # ---- persistent SBUF ----
        xT_s = sb.tile([P, EC, S], bf16)    # x[b]^T  (e on partitions)
        xqT_s = sb.tile([P, EC, D], bf16)   # own-query columns of x^T
        KT_s = sb.tile([P, EC, S], bf16)    # K^T (d on partitions)
        V_s = sb.tile([P, NKB, D], bf16)    # V natural (s on partitions)
        QT_s = sb.tile([P, EC, D], bf16)    # Q^T for own queries
        mask_s = sb.tile([P, NQB, 2 * P], f32)
        ident = sb.tile([P, P], bf16)
        make_identity(nc, ident)

        def load_w(w_dram, nm):
            w_s = sb.tile([P, EC, D], bf16, tag="w", bufs=2, name=nm)
            for ec in range(EC):
                nc.sync.dma_start(w_s[:, ec], w_dram[ts(ec, P), :])
            return w_s

        # interleave the first projection's operands so PE starts ASAP
        wq_s = sb.tile([P, EC, D], bf16, tag="w", bufs=2, name="wq_s")
        for ec in range(EC):
            nc.sync.dma_start(wq_s[:, ec], wqT[ts(ec, P), :])
            nc.sync.dma_start(xqT_s[:, ec], xqT[ts(ec, P), :])
        for ec in range(EC):
            nc.sync.dma_start(xT_s[:, ec], xT[ts(ec, P), :])
        for j in range(NQB):
            nc.sync.dma_start(mask_s[:, j], maskadd[j])

        # ---- Q^T projection: QT[d, q] = sum_e WqT[e, d] * xqT[e, q] ----
        for dc in range(EC):
            acc = ps.tile([P, D], f32, tag="big")
            for ec in range(EC):
                lhsT = wq_s[:, ec, ts(dc, P)]
                for nh in range(2):
                    nc.tensor.matmul(
                        acc[:, ts(nh, 512)], lhsT, xqT_s[:, ec, ts(nh, 512)],
                        start=(ec == 0), stop=(ec == EC - 1))
            nc.scalar.copy(QT_s[:, dc], acc)

        # ---- K^T projection, s-chunk-major so early key columns finish first ----
        wk_s = load_w(wkT, "wk_s")
        for sc in range(S // 512):
            for dc in range(EC):
                acc = ps.tile([P, 512], f32, tag="s")
                for ec in range(EC):
                    nc.tensor.matmul(
                        acc, wk_s[:, ec, ts(dc, P)], xT_s[:, ec, ts(sc, 512)],
                        start=(ec == 0), stop=(ec == EC - 1))
                nc.scalar.copy(KT_s[:, dc, ts(sc, 512)], acc)

        # ---- V projection: V[s, d] = sum_e xT[e, s] * WvT[e, d] ----
        wv_s = load_w(wvT, "wv_s")
        for kb in range(NKB):
            acc = ps.tile([P, D], f32, tag="big")
            for ec in range(EC):
                lhsT = xT_s[:, ec, ts(kb, P)]
                for nh in range(2):
                    nc.tensor.matmul(
                        acc[:, ts(nh, 512)], lhsT, wv_s[:, ec, ts(nh, 512)],
                        start=(ec == 0), stop=(ec == EC - 1))
            nc.vector.tensor_copy(V_s[:, kb], acc)

        # ---- attention, one 128-row query block at a time ----
        # Software-pipelined: S/exp of the NEXT block is traced between the
        # S/exp and transpose/AV of the current one, so the PE has matmul work
        # while ACT/DVE chew through exp and P^T copies.
        inv_sqrt_d = 1.0 / float(np.sqrt(D))

        def s_phase(j):
            nkt = 2 * j + 2          # key tiles (uniform across cores)
            ncols = nkt * P
            nch = (ncols + 511) // 512
            p_sb = sb.tile([P, S], bf16, tag="p_sb", bufs=2)
            dsl = sb.tile([P, 4], f32, tag="dsl", bufs=2)
            for ch in range(nch):
                c0 = ch * 512
                cw = min(512, ncols - c0)
                sfull = ps.tile([P, 512], f32, tag="s")
                sps = sfull[:, :cw]
                for dc in range(EC):
                    nc.tensor.matmul(
                        sps, QT_s[:, dc, ts(j, P)], KT_s[:, dc, c0:c0 + cw],
                        start=(dc == 0), stop=(dc == EC - 1))
                if c0 + cw == ncols:  # last chunk holds the 2 maskable tiles
                    nc.vector.tensor_add(
                        sps[:, cw - 2 * P:cw], sps[:, cw - 2 * P:cw], mask_s[:, j])
                nc.scalar.activation(
                    p_sb[:, c0:c0 + cw], sps,
                    mybir.ActivationFunctionType.Exp,
                    scale=inv_sqrt_d,
                    accum_out=dsl[:, ch:ch + 1])
            return p_sb, dsl, nkt, nch

        def av_phase(j, p_sb, dsl, nkt, nch):
            denom = sb.tile([P, 1], f32, tag="den", bufs=2)
            nc.vector.reduce_sum(denom, dsl[:, :nch], axis=mybir.AxisListType.X)
            recip = sb.tile([P, 1], f32, tag="rcp", bufs=2)
            nc.vector.reciprocal(recip, denom)

            pts = []
            for kt in range(nkt):
                ptp = ps.tile([P, P], bf16, tag="pt")
                nc.tensor.transpose(ptp, p_sb[:, ts(kt, P)], ident)
                pt_sb = sb.tile([P, P], bf16, tag="pt_sb", bufs=16)
                nc.vector.tensor_copy(pt_sb, ptp)
                pts.append(pt_sb)

            acc = ps.tile([P, D], f32, tag="big")
            for kt in range(nkt):
                for nh in range(2):
                    nc.tensor.matmul(
                        acc[:, ts(nh, 512)], pts[kt], V_s[:, kt, ts(nh, 512)],
                        start=(kt == 0), stop=(kt == nkt - 1))
            o_sb = sb.tile([P, D], f32, tag="o_sb", bufs=2)
            # normalize on ACT (idle here) so DVE stays free for PT copies
            nc.scalar.activation(o_sb, acc, mybir.ActivationFunctionType.Copy,
                                 scale=recip)
            nc.sync.dma_start(out[j], o_sb)

        order = list(reversed(range(NQB)))  # big first: shortest tail ends kernel
        pending = None
        for j in order:
            state = s_phase(j)
            if pending is not None:
                av_phase(*pending)
            pending = (j,) + state
        av_phase(*pending)


def build_nc(debug=False):
    nc = bacc.Bacc("TRN2", target_bir_lowering=False, debug=debug,
                   enable_asserts=False, num_devices=NCORES)
    bf16 = mybir.dt.bfloat16
    f32 = mybir.dt.float32
    xT = nc.dram_tensor("xT", (D, S), bf16, kind="ExternalInput").ap()
    xqT = nc.dram_tensor("xqT", (D, D), bf16, kind="ExternalInput").ap()
    wqT = nc.dram_tensor("wqT", (D, D), bf16, kind="ExternalInput").ap()
    wkT = nc.dram_tensor("wkT", (D, D), bf16, kind="ExternalInput").ap()
    wvT = nc.dram_tensor("wvT", (D, D), bf16, kind="ExternalInput").ap()
    maskadd = nc.dram_tensor("maskadd", (NQB, P, 2 * P), f32,
                             kind="ExternalInput").ap()
    out = nc.dram_tensor("out", (NQB, P, D), f32, kind="ExternalOutput").ap()
    with tile.TileContext(nc) as tc:
        _trace_kernel(tc, out, xT, xqT, wqT, wkT, wvT, maskadd)
    nc.compile()
    return nc


def _get_compiled():
    global _compiled_nc
    if _compiled_nc is None:
        _compiled_nc = build_nc(debug=False)
    return _compiled_nc


def _get_runner():
    """Jit-once shard_map runner over the 8 NeuronCores.

    Mirrors bass2jax.run_bass_via_pjrt's multi-core branch, but caches the
    jitted executable so repeat kernel() calls skip retracing/recompiling.
    """
    global _runner
    if _runner is not None:
        return _runner
    import jax
    from jax.experimental.shard_map import shard_map
    from jax.sharding import Mesh, PartitionSpec
    from concourse import bass2jax

    nc = _get_compiled()
    bass2jax.install_neuronx_cc_hook()

    partition_name = (nc.partition_id_tensor.name
                      if nc.partition_id_tensor else None)
    in_names, out_names, out_avals, zero_outs = [], [], [], []
    for alloc in nc.m.functions[0].allocations:
        if not isinstance(alloc, mybir.MemoryLocationSet):
            continue
        name = alloc.memorylocations[0].name
        if alloc.kind == "ExternalInput":
            if name != partition_name:
                in_names.append(name)
        elif alloc.kind == "ExternalOutput":
            shape = tuple(alloc.tensor_shape)
            dtype = mybir.dt.np(alloc.dtype)
            out_names.append(name)
            out_avals.append(jax.core.ShapedArray(shape, dtype))
            zero_outs.append(np.zeros(shape, dtype))
    n_params = len(in_names)
    all_in_names = list(in_names) + list(out_names)
    if partition_name is not None:
        all_in_names.append(partition_name)
    donate = tuple(range(n_params, n_params + len(out_names)))

    def _body(*args):
        operands = list(args)
        if partition_name is not None:
            operands.append(bass2jax.partition_id_tensor())
        outs = bass2jax._bass_exec_p.bind(
            *operands,
            out_avals=tuple(out_avals),
            in_names=tuple(all_in_names),
            out_names=tuple(out_names),
            lowering_input_output_aliases=(),
            sim_require_finite=True,
            sim_require_nnan=True,
            nc=nc,
        )
        return tuple(outs)

    devices = jax.devices()[:NCORES]
    mesh = Mesh(np.asarray(devices), ("core",))
    nin = n_params + len(out_names)
    sharded = jax.jit(
        shard_map(_body, mesh=mesh,
                  in_specs=(PartitionSpec("core"),) * nin,
                  out_specs=(PartitionSpec("core"),) * len(out_names),
                  check_rep=False),
        donate_argnums=donate, keep_unused=True)
    _runner = (sharded, in_names, out_names, out_avals, n_params, zero_outs, mesh)
    return _runner


def run_device(in_maps):
    """Execute the compiled NEFF on all 8 cores; returns per-core output dicts."""
    sharded, in_names, out_names, out_avals, n_params, zero_outs, _ = _get_runner()
    concat_in = [
        np.concatenate([np.asarray(in_maps[c][nm]) for c in range(NCORES)], axis=0)
        for nm in in_names
    ]
    concat_zeros = [
        np.zeros((NCORES * z.shape[0], *z.shape[1:]), z.dtype) for z in zero_outs
    ]
    out_arrs = sharded(*concat_in, *concat_zeros)
    return [
        {nm: np.asarray(out_arrs[i]).reshape(NCORES, *out_avals[i].shape)[c]
         for i, nm in enumerate(out_names)}
        for c in range(NCORES)
    ]


def make_in_maps(x):
    """Per-core host-side slicing + layout prep (no matmul math here)."""
    x = np.asarray(x, dtype=np.float32)
    r = np.arange(P)
    tri_add = np.where(r[None, :] <= r[:, None], 0.0, MASKV).astype(np.float32)
    mask_h = []
    for h in range(2):
        if h == 0:
            blk = np.concatenate(
                [tri_add, np.full((P, P), MASKV, np.float32)], axis=1)
        else:
            blk = np.concatenate([np.zeros((P, P), np.float32), tri_add], axis=1)
        mask_h.append(np.ascontiguousarray(
            np.broadcast_to(blk, (NQB, P, 2 * P))).astype(np.float32))

    in_maps = []
    xT_b = {}
    for c in range(NCORES):
        b, h = c // 2, c % 2
        if b not in xT_b:
            xT_b[b] = np.ascontiguousarray(x[b].T).astype(BF16)
        blocks = [2 * j + h for j in range(NQB)]
        xq = np.concatenate([x[b][g * P:(g + 1) * P] for g in blocks], axis=0)
        xqT = np.ascontiguousarray(xq.T).astype(BF16)
        in_maps.append({
            "xT": xT_b[b],
            "xqT": xqT,
            "maskadd": mask_h[h],
        })
    return in_maps


def kernel(x, Wq, bq, Wk, bk, Wv, bv, mask):
    global last_result
    x = np.asarray(x, np.float32)
    Wq = np.asarray(Wq, np.float32)
    Wk = np.asarray(Wk, np.float32)
    Wv = np.asarray(Wv, np.float32)
    bq = np.asarray(bq, np.float32)
    bk = np.asarray(bk, np.float32)
    bv = np.asarray(bv, np.float32)
    mask = np.asarray(mask)

    causal = bool(np.array_equal(mask != 0, np.tril(np.ones(mask.shape, bool))))
    if np.any(bq) or np.any(bk) or not causal:
        return _np_reference(x, Wq, bq, Wk, bk, Wv, bv, mask)

    in_maps = make_in_maps(x)
    wT = {
        "wqT": np.ascontiguousarray(Wq.T).astype(BF16),
        "wkT": np.ascontiguousarray(Wk.T).astype(BF16),
        "wvT": np.ascontiguousarray(Wv.T).astype(BF16),
    }
    for m in in_maps:
        m.update(wT)

    results = run_device(in_maps)

    out = np.empty((B * S, D), np.float32)
    for c in range(NCORES):
        b, h = c // 2, c % 2
        o = np.asarray(results[c]["out"], np.float32)
        for j in range(NQB):
            g = 2 * j + h
            out[b * S + g * P: b * S + (g + 1) * P] = o[j]
    if np.any(bv):
        out = out + bv[None, :]  # attn rows sum to 1, so bv adds exactly
    return out


def _np_reference(x, Wq, bq, Wk, bk, Wv, bv, mask):
    outs = []
    for b in range(x.shape[0]):
        xb = x[b]
        Q = xb @ Wq.T + bq
        K = xb @ Wk.T + bk
        V = xb @ Wv.T + bv
        Sc = (Q @ K.T) / np.float32(np.sqrt(x.shape[2]))
        Sc = np.where(mask == 0, np.float32(-1e9), Sc)
        Sc = Sc - Sc.max(axis=1, keepdims=True)
        E = np.exp(Sc)
        A = E / E.sum(axis=1, keepdims=True)
        outs.append(A @ V)
    return np.concatenate(outs, axis=0).astype(np.float32)


# revision 18
# speedup vs baseline: 680.4559x; 1.2240x over previous
"""Causal self-attention (B=4, S=2048, D=1024, single 1024-wide head) on 8 TRN2 cores.

Sharding: core c -> batch b=c//2, parity h=c%2. Each core computes K/V for its
whole batch (duplicated across the 2 cores of a batch) and handles the 8
query blocks {h, h+2, ..., h+14} (128 rows each). Pairing strided blocks keeps
causal work balanced and — with key-extents padded to 256*(j+1) — makes the
program identical on every core; causality differences live in per-core
additive-mask input data, not control flow.

All matmuls run on the PE in bf16 with fp32 PSUM accumulation. Softmax skips
max-subtraction (scores are ~N(0,1); exp stays in fp32 range) so the
denominator comes free from the Exp activation's accumulate output.
"""

import numpy as np
import ml_dtypes

import concourse.bass as bass
import concourse.bacc as bacc
import concourse.tile as tile
from concourse import mybir
from concourse import bass_utils
from concourse.masks import make_identity

BF16 = ml_dtypes.bfloat16
P = 128
B, S, D = 4, 2048, 1024
EC = D // P  # contraction chunks (8)
NQB = 8      # query blocks per core
NKB = S // P  # key blocks per batch (16)
NCORES = 8
MASKV = -960.0  # additive pre-scale mask; -30 after the 1/sqrt(D) scale

_compiled_nc = None
_runner = None  # cached (sharded_jit, in_names, out_names, out_avals, n_params)
last_result = None  # kept for compatibility with older test harnesses


def _trace_kernel(tc, out, xT, xqT, wqT, wkT, wvT, maskadd):
    nc = tc.nc
    f32 = mybir.dt.float32
    bf16 = mybir.dt.bfloat16
    ts = bass.ts

    with (
        tc.tile_pool(name="sb", bufs=1) as sb,
        tc.tile_pool(name="ps", bufs=2, space="PSUM") as ps,
    ):
        # ---- persistent SBUF ----
        xT_s = sb.tile([P, EC, S], bf16)    # x[b]^T  (e on partitions)
        xqT_s = sb.tile([P, EC, D], bf16)   # own-query columns of x^T
        KT_s = sb.tile([P, EC, S], bf16)    # K^T (d on partitions)
        V_s = sb.tile([P, NKB, D], bf16)    # V natural (s on partitions)
        QT_s = sb.tile([P, EC, D], bf16)    # Q^T for own queries
        mask_s = sb.tile([P, NQB, 2 * P], f32)
        ident = sb.tile([P, P], bf16)
        make_identity(nc, ident)

        def load_w(w_dram, nm):
            w_s = sb.tile([P, EC, D], bf16, tag="w", bufs=2, name=nm)
            for ec in range(EC):
                nc.sync.dma_start(w_s[:, ec], w_dram[ts(ec, P), :])
            return w_s

        # interleave the first projection's operands so PE starts ASAP
        wq_s = sb.tile([P, EC, D], bf16, tag="w", bufs=2, name="wq_s")
        for ec in range(EC):
            nc.sync.dma_start(wq_s[:, ec], wqT[ts(ec, P), :])
            nc.sync.dma_start(xqT_s[:, ec], xqT[ts(ec, P), :])
        for ec in range(EC):
            nc.sync.dma_start(xT_s[:, ec], xT[ts(ec, P), :])
        for j in range(NQB):
            nc.sync.dma_start(mask_s[:, j], maskadd[j])

        # ---- Q^T projection: QT[d, q] = sum_e WqT[e, d] * xqT[e, q] ----
        for dc in range(EC):
            acc = ps.tile([P, D], f32, tag="big")
            for ec in range(EC):
                lhsT = wq_s[:, ec, ts(dc, P)]
                for nh in range(2):
                    nc.tensor.matmul(
                        acc[:, ts(nh, 512)], lhsT, xqT_s[:, ec, ts(nh, 512)],
                        start=(ec == 0), stop=(ec == EC - 1))
            nc.scalar.copy(QT_s[:, dc], acc)

        # ---- K^T projection, s-chunk-major so early key columns finish first ----
        wk_s = load_w(wkT, "wk_s")
        for sc in range(S // 512):
            for dc in range(EC):
                acc = ps.tile([P, 512], f32, tag="s")
                for ec in range(EC):
                    nc.tensor.matmul(
                        acc, wk_s[:, ec, ts(dc, P)], xT_s[:, ec, ts(sc, 512)],
                        start=(ec == 0), stop=(ec == EC - 1))
                nc.scalar.copy(KT_s[:, dc, ts(sc, 512)], acc)

        # ---- V projection: V[s, d] = sum_e xT[e, s] * WvT[e, d] ----
        wv_s = load_w(wvT, "wv_s")
        for kb in range(NKB):
            acc = ps.tile([P, D], f32, tag="big")
            for ec in range(EC):
                lhsT = xT_s[:, ec, ts(kb, P)]
                for nh in range(2):
                    nc.tensor.matmul(
                        acc[:, ts(nh, 512)], lhsT, wv_s[:, ec, ts(nh, 512)],
                        start=(ec == 0), stop=(ec == EC - 1))
            nc.vector.tensor_copy(V_s[:, kb], acc)

        # ---- attention, one 128-row query block at a time ----
        # Software-pipelined: S/exp of the NEXT block is traced between the
        # S/exp and transpose/AV of the current one, so the PE has matmul work
        # while ACT/DVE chew through exp and P^T copies.
        inv_sqrt_d = 1.0 / float(np.sqrt(D))

        def s_phase(j):
            nkt = 2 * j + 2          # key tiles (uniform across cores)
            ncols = nkt * P
            nch = (ncols + 511) // 512
            p_sb = sb.tile([P, S], bf16, tag="p_sb", bufs=2)
            dsl = sb.tile([P, 4], f32, tag="dsl", bufs=2)
            for ch in range(nch):
                c0 = ch * 512
                cw = min(512, ncols - c0)
                sfull = ps.tile([P, 512], f32, tag="s")
                sps = sfull[:, :cw]
                for dc in range(EC):
                    nc.tensor.matmul(
                        sps, QT_s[:, dc, ts(j, P)], KT_s[:, dc, c0:c0 + cw],
                        start=(dc == 0), stop=(dc == EC - 1))
                if c0 + cw == ncols:  # last chunk holds the 2 maskable tiles
                    nc.vector.tensor_add(
                        sps[:, cw - 2 * P:cw], sps[:, cw - 2 * P:cw], mask_s[:, j])
                nc.scalar.activation(
                    p_sb[:, c0:c0 + cw], sps,
                    mybir.ActivationFunctionType.Exp,
                    scale=inv_sqrt_d,
                    accum_out=dsl[:, ch:ch + 1])
            return p_sb, dsl, nkt, nch

        def av_phase(j, p_sb, dsl, nkt, nch, dve_norm=False):
            denom = sb.tile([P, 1], f32, tag="den", bufs=2)
            nc.vector.reduce_sum(denom, dsl[:, :nch], axis=mybir.AxisListType.X)
            recip = sb.tile([P, 1], f32, tag="rcp", bufs=2)
            nc.vector.reciprocal(recip, denom)

            pts = []
            for kt in range(nkt):
                ptp = ps.tile([P, P], bf16, tag="pt")
                nc.tensor.transpose(ptp, p_sb[:, ts(kt, P)], ident)
                pt_sb = sb.tile([P, P], bf16, tag="pt_sb", bufs=16)
                nc.vector.tensor_copy(pt_sb, ptp)
                pts.append(pt_sb)

            acc = ps.tile([P, D], f32, tag="big")
            for kt in range(nkt):
                for nh in range(2):
                    nc.tensor.matmul(
                        acc[:, ts(nh, 512)], pts[kt], V_s[:, kt, ts(nh, 512)],
                        start=(kt == 0), stop=(kt == nkt - 1))
            o_sb = sb.tile([P, D], f32, tag="o_sb", bufs=2)
            if dve_norm:
                # tail block: DVE is idle and skips the ACT table swap
                nc.vector.tensor_scalar_mul(o_sb, acc, recip)
            else:
                # normalize on ACT (idle here) so DVE stays free for PT copies
                nc.scalar.activation(o_sb, acc,
                                     mybir.ActivationFunctionType.Copy,
                                     scale=recip)
            nc.sync.dma_start(out[j], o_sb)

        order = list(reversed(range(NQB)))  # big first: shortest tail ends kernel
        pending = None
        for j in order:
            state = s_phase(j)
            if pending is not None:
                av_phase(*pending)
            pending = (j,) + state
        av_phase(*pending, dve_norm=True)


def build_nc(debug=False):
    nc = bacc.Bacc("TRN2", target_bir_lowering=False, debug=debug,
                   enable_asserts=False, num_devices=NCORES)
    bf16 = mybir.dt.bfloat16
    f32 = mybir.dt.float32
    xT = nc.dram_tensor("xT", (D, S), bf16, kind="ExternalInput").ap()
    xqT = nc.dram_tensor("xqT", (D, D), bf16, kind="ExternalInput").ap()
    wqT = nc.dram_tensor("wqT", (D, D), bf16, kind="ExternalInput").ap()
    wkT = nc.dram_tensor("wkT", (D, D), bf16, kind="ExternalInput").ap()
    wvT = nc.dram_tensor("wvT", (D, D), bf16, kind="ExternalInput").ap()
    maskadd = nc.dram_tensor("maskadd", (NQB, P, 2 * P), f32,
                             kind="ExternalInput").ap()
    out = nc.dram_tensor("out", (NQB, P, D), f32, kind="ExternalOutput").ap()
    with tile.TileContext(nc) as tc:
        _trace_kernel(tc, out, xT, xqT, wqT, wkT, wvT, maskadd)
    nc.compile()
    return nc


def _get_compiled():
    global _compiled_nc
    if _compiled_nc is None:
        _compiled_nc = build_nc(debug=False)
    return _compiled_nc


def _get_runner():
    """Jit-once shard_map runner over the 8 NeuronCores.

    Mirrors bass2jax.run_bass_via_pjrt's multi-core branch, but caches the
    jitted executable so repeat kernel() calls skip retracing/recompiling.
    """
    global _runner
    if _runner is not None:
        return _runner
    import jax
    from jax.experimental.shard_map import shard_map
    from jax.sharding import Mesh, PartitionSpec
    from concourse import bass2jax

    nc = _get_compiled()
    bass2jax.install_neuronx_cc_hook()

    partition_name = (nc.partition_id_tensor.name
                      if nc.partition_id_tensor else None)
    in_names, out_names, out_avals, zero_outs = [], [], [], []
    for alloc in nc.m.functions[0].allocations:
        if not isinstance(alloc, mybir.MemoryLocationSet):
            continue
        name = alloc.memorylocations[0].name
        if alloc.kind == "ExternalInput":
            if name != partition_name:
                in_names.append(name)
        elif alloc.kind == "ExternalOutput":
            shape = tuple(alloc.tensor_shape)
            dtype = mybir.dt.np(alloc.dtype)
            out_names.append(name)
            out_avals.append(jax.core.ShapedArray(shape, dtype))
            zero_outs.append(np.zeros(shape, dtype))
    n_params = len(in_names)
    all_in_names = list(in_names) + list(out_names)
    if partition_name is not None:
        all_in_names.append(partition_name)
    donate = tuple(range(n_params, n_params + len(out_names)))

    def _body(*args):
        operands = list(args)
        if partition_name is not None:
            operands.append(bass2jax.partition_id_tensor())
        outs = bass2jax._bass_exec_p.bind(
            *operands,
            out_avals=tuple(out_avals),
            in_names=tuple(all_in_names),
            out_names=tuple(out_names),
            lowering_input_output_aliases=(),
            sim_require_finite=True,
            sim_require_nnan=True,
            nc=nc,
        )
        return tuple(outs)

    devices = jax.devices()[:NCORES]
    mesh = Mesh(np.asarray(devices), ("core",))
    nin = n_params + len(out_names)
    sharded = jax.jit(
        shard_map(_body, mesh=mesh,
                  in_specs=(PartitionSpec("core"),) * nin,
                  out_specs=(PartitionSpec("core"),) * len(out_names),
                  check_rep=False),
        donate_argnums=donate, keep_unused=True)
    _runner = (sharded, in_names, out_names, out_avals, n_params, zero_outs, mesh)
    return _runner


def run_device(in_maps):
    """Execute the compiled NEFF on all 8 cores; returns per-core output dicts."""
    sharded, in_names, out_names, out_avals, n_params, zero_outs, _ = _get_runner()
    concat_in = [
        np.concatenate([np.asarray(in_maps[c][nm]) for c in range(NCORES)], axis=0)
        for nm in in_names
    ]
    concat_zeros = [
        np.zeros((NCORES * z.shape[0], *z.shape[1:]), z.dtype) for z in zero_outs
    ]
    out_arrs = sharded(*concat_in, *concat_zeros)
    return [
        {nm: np.asarray(out_arrs[i]).reshape(NCORES, *out_avals[i].shape)[c]
         for i, nm in enumerate(out_names)}
        for c in range(NCORES)
    ]


def make_in_maps(x):
    """Per-core host-side slicing + layout prep (no matmul math here)."""
    x = np.asarray(x, dtype=np.float32)
    r = np.arange(P)
    tri_add = np.where(r[None, :] <= r[:, None], 0.0, MASKV).astype(np.float32)
    mask_h = []
    for h in range(2):
        if h == 0:
            blk = np.concatenate(
                [tri_add, np.full((P, P), MASKV, np.float32)], axis=1)
        else:
            blk = np.concatenate([np.zeros((P, P), np.float32), tri_add], axis=1)
        mask_h.append(np.ascontiguousarray(
            np.broadcast_to(blk, (NQB, P, 2 * P))).astype(np.float32))

    in_maps = []
    xT_b = {}
    for c in range(NCORES):
        b, h = c // 2, c % 2
        if b not in xT_b:
            xT_b[b] = np.ascontiguousarray(x[b].T).astype(BF16)
        blocks = [2 * j + h for j in range(NQB)]
        xq = np.concatenate([x[b][g * P:(g + 1) * P] for g in blocks], axis=0)
        xqT = np.ascontiguousarray(xq.T).astype(BF16)
        in_maps.append({
            "xT": xT_b[b],
            "xqT": xqT,
            "maskadd": mask_h[h],
        })
    return in_maps


def kernel(x, Wq, bq, Wk, bk, Wv, bv, mask):
    global last_result
    x = np.asarray(x, np.float32)
    Wq = np.asarray(Wq, np.float32)
    Wk = np.asarray(Wk, np.float32)
    Wv = np.asarray(Wv, np.float32)
    bq = np.asarray(bq, np.float32)
    bk = np.asarray(bk, np.float32)
    bv = np.asarray(bv, np.float32)
    mask = np.asarray(mask)

    causal = bool(np.array_equal(mask != 0, np.tril(np.ones(mask.shape, bool))))
    if np.any(bq) or np.any(bk) or not causal:
        return _np_reference(x, Wq, bq, Wk, bk, Wv, bv, mask)

    in_maps = make_in_maps(x)
    wT = {
        "wqT": np.ascontiguousarray(Wq.T).astype(BF16),
        "wkT": np.ascontiguousarray(Wk.T).astype(BF16),
        "wvT": np.ascontiguousarray(Wv.T).astype(BF16),
    }
    for m in in_maps:
        m.update(wT)

    results = run_device(in_maps)

    out = np.empty((B * S, D), np.float32)
    for c in range(NCORES):
        b, h = c // 2, c % 2
        o = np.asarray(results[c]["out"], np.float32)
        for j in range(NQB):
            g = 2 * j + h
            out[b * S + g * P: b * S + (g + 1) * P] = o[j]
    if np.any(bv):
        out = out + bv[None, :]  # attn rows sum to 1, so bv adds exactly
    return out


def _np_reference(x, Wq, bq, Wk, bk, Wv, bv, mask):
    outs = []
    for b in range(x.shape[0]):
        xb = x[b]
        Q = xb @ Wq.T + bq
        K = xb @ Wk.T + bk
        V = xb @ Wv.T + bv
        Sc = (Q @ K.T) / np.float32(np.sqrt(x.shape[2]))
        Sc = np.where(mask == 0, np.float32(-1e9), Sc)
        Sc = Sc - Sc.max(axis=1, keepdims=True)
        E = np.exp(Sc)
        A = E / E.sum(axis=1, keepdims=True)
        outs.append(A @ V)
    return np.concatenate(outs, axis=0).astype(np.float32)
